# revision 1
# baseline (speedup 1.0000x reference)
"""Gumbel-softmax sample + symmetric scatter kernel for 8 trn2 NeuronCores.

Math: out[e] = sigmoid(((g0 - g1) + (gum0 - gum1)) / TEMP) with
gum_k = -log(-log(u_k + EPS) + EPS).  The scatter target is fully
deterministic: part 1 (first S*DEL_NUM elements) is a dense [S, DEL_NUM]
block at matrix[0:S, S:SZ]; part 2 is the strict upper triangle of the
bottom-right [DEL_NUM, DEL_NUM] block.  Output = matrix + matrix.T.

Device: each core computes a contiguous 1/8 of the E sigmoid values
(memory-bound elementwise map).  Host places the values into the
symmetric [SZ, SZ] output.
"""

import numpy as np

SZ = 8192
DEL_NUM = 2048
S = SZ - DEL_NUM               # 6144
E1 = S * DEL_NUM               # 12,582,912 dense block elements
E2 = DEL_NUM * (DEL_NUM - 1) // 2  # 2,096,128 triangular elements
E = E1 + E2                    # 14,679,040
NCORES = 8
CH = E // NCORES               # 1,834,880 elements per core
P = 128
FTOT = CH // P                 # 14,335 outputs per partition
NCHUNK = 5
F = FTOT // NCHUNK             # 2,867
TEMP = 10.0
EPS = 1e-20

_cache = {}

# Sigmoid placement: "batch" = all sigmoids after all Ln passes (2 ACT
# table loads total); "inline" = sigmoid right after each chunk's combine
# (2 loads per chunk, but output DMAs overlap input stream).
VARIANT = "batch"


def _build(variant=None):
    import concourse.bacc as bacc
    import concourse.mybir as mybir
    import concourse.tile as tile

    f32 = mybir.dt.float32
    AF = mybir.ActivationFunctionType

    nc = bacc.Bacc(
        "TRN2", target_bir_lowering=False, debug=False, num_devices=NCORES
    )

    # Float activation biases require registered const APs.
    for val in (EPS,):
        t = nc.alloc_sbuf_tensor(f"const-f32-{val}", [128, 1], f32)
        nc.gpsimd.memset(t.ap(), val)
        nc.const_aps.aps[(f32, val)] = t.ap()
    nc.all_engine_barrier()

    gen_ap = nc.dram_tensor("gen", [P, 2 * FTOT], f32, kind="ExternalInput").ap()
    u_ap = nc.dram_tensor("u", [P, 2 * FTOT], f32, kind="ExternalInput").ap()
    out_ap = nc.dram_tensor("out", [P, FTOT], f32, kind="ExternalOutput").ap()

    if variant is None:
        variant = VARIANT
    # "tail" variant: big chunks first, tiny last chunk -> the serial
    # chain behind the last-arriving input DMA (Ln,Ln,DVE*3,table
    # switch,Sigmoid,out-DMA) shrinks from ~25us to ~6us.  Big chunks'
    # sigmoids run before the last chunk's Lns so the final sigmoid
    # table load hides under the last chunk's DVE work.
    if variant == "tail":
        sizes = [3456, 3456, 3456, 3456, 511]
    else:
        sizes = [F] * NCHUNK
    offs = [sum(sizes[:i]) for i in range(len(sizes))]
    with tile.TileContext(nc) as tc:
        with tc.tile_pool(name="pool", bufs=2) as pool:
            s_tiles = []
            for i in range(NCHUNK):
                Fi, Oi = sizes[i], offs[i]
                fs = slice(2 * Oi, 2 * (Oi + Fi))
                ut = pool.tile([P, 2 * Fi], f32, tag="u", bufs=3 if variant == "buf3" else 2)
                nc.sync.dma_start(ut[:], u_ap[:, fs])
                gt = pool.tile([P, 2 * Fi], f32, tag="g", bufs=2)
                nc.sync.dma_start(gt[:], gen_ap[:, fs])

                # l1 = log(u + eps); w = max(-l1, 2^-24) (guards LUT error at
                # u ~= 1 from sending a <=0 value into the second log);
                # l2 = log(w) = -gumbel.  All in place in ut.
                nc.scalar.activation(ut[:], ut[:], AF.Ln, bias=EPS)
                nc.vector.tensor_scalar(
                    ut[:], ut[:], -1.0, 5.9604645e-08,
                    op0=mybir.AluOpType.mult, op1=mybir.AluOpType.max,
                )
                nc.scalar.activation(ut[:], ut[:], AF.Ln, bias=0.0)

                lv = ut.rearrange("p (f two) -> p f two", two=2)
                gv = gt.rearrange("p (f two) -> p f two", two=2)
                s = pool.tile([P, Fi], f32, tag="s", bufs=NCHUNK)
                # s = (g0 - g1) + l2_1 - l2_0  (gum0 - gum1 = l2_1 - l2_0)
                nc.vector.tensor_sub(s[:], gv[:, :, 0], gv[:, :, 1])
                nc.vector.tensor_add(s[:], s[:], lv[:, :, 1])
                nc.vector.tensor_sub(s[:], s[:], lv[:, :, 0])
                if variant == "inline":
                    nc.scalar.activation(s[:], s[:], AF.Sigmoid, scale=1.0 / TEMP)
                    nc.sync.dma_start(out_ap[:, Oi : Oi + Fi], s[:])
                else:
                    s_tiles.append((s, Oi, Fi))

                # "split"/"tail": drain ready sigmoids before the last chunk
                # so the final table switch is off the critical tail.
                if variant in ("split", "tail") and i == NCHUNK - 2:
                    for sj, Oj, Fj in s_tiles:
                        nc.scalar.activation(sj[:], sj[:], AF.Sigmoid, scale=1.0 / TEMP)
                        nc.sync.dma_start(out_ap[:, Oj : Oj + Fj], sj[:])
                    s_tiles = []

            # Sigmoids batched last: one Ln->Sigmoid ACT table switch total.
            # "ring2": output DMAs ride the second HWDGE ring (qActDynamicHW)
            # so they interleave with the input stream at SDMA level.
            out_eng = nc.scalar if variant == "ring2" else nc.sync
            for s, Oi, Fi in s_tiles:
                nc.scalar.activation(s[:], s[:], AF.Sigmoid, scale=1.0 / TEMP)
                out_eng.dma_start(out_ap[:, Oi : Oi + Fi], s[:])

    nc.compile()
    return nc


def get_nc(variant=None):
    key = variant or VARIANT
    if key not in _cache:
        _cache[key] = _build(key)
    return _cache[key]


def run_cores(gen: np.ndarray, u: np.ndarray, trace: bool = False):
    """Run the SPMD kernel on flat [E, 2] inputs; returns (flat out [E], results obj)."""
    from concourse.bass_utils import run_bass_kernel_spmd

    nc = get_nc()
    in_maps = []
    for c in range(NCORES):
        sl = slice(c * CH, (c + 1) * CH)
        in_maps.append(
            {
                "gen": gen[sl].reshape(P, 2 * FTOT),
                "u": u[sl].reshape(P, 2 * FTOT),
            }
        )
    kw = {}
    if trace:
        kw = {"trace": True, "trace_cores": list(range(NCORES)), "stitch_traces": True}
    res = run_bass_kernel_spmd(nc, in_maps, core_ids=list(range(NCORES)), **kw)
    out = np.concatenate([np.asarray(r["out"]).reshape(-1) for r in res.results])
    return out, res


def assemble(out: np.ndarray) -> np.ndarray:
    full = np.zeros((SZ, SZ), np.float32)
    a = out[:E1].reshape(S, DEL_NUM)
    full[:S, S:] = a
    full[S:, :S] = a.T
    ti, tj = np.triu_indices(DEL_NUM, k=1)
    b = np.zeros((DEL_NUM, DEL_NUM), np.float32)
    b[ti, tj] = out[E1:]
    full[S:, S:] = b + b.T
    return full


def kernel(gen_matrix=None, u=None, sz=None, del_num=None, **_ignored):
    gen = np.ascontiguousarray(np.asarray(gen_matrix, dtype=np.float32))
    uu = np.ascontiguousarray(np.asarray(u, dtype=np.float32))
    assert gen.shape == (E, 2) and uu.shape == (E, 2)
    out, _ = run_cores(gen, uu)
    return assemble(out)



# revision 3
# speedup vs baseline: 10.5924x; 10.5924x over previous
"""Gumbel-softmax sample + symmetric scatter kernel for 8 trn2 NeuronCores.

Math: out[e] = sigmoid(((g0 - g1) + (gum0 - gum1)) / TEMP) with
gum_k = -log(-log(u_k + EPS) + EPS).  The scatter target is fully
deterministic: part 1 (first S*DEL_NUM elements) is a dense [S, DEL_NUM]
block at matrix[0:S, S:SZ]; part 2 is the strict upper triangle of the
bottom-right [DEL_NUM, DEL_NUM] block.  Output = matrix + matrix.T.

Device: each core computes a contiguous 1/8 of the E sigmoid values
(memory-bound elementwise map).  Host places the values into the
symmetric [SZ, SZ] output.

v3 design (per core, per chunk; all engine operands f16 so DVE runs in
its 2x/4x perf modes and HBM traffic is halved):
  ACT:  w' = Ln(u + EPS)                   (f16 in -> f16 out)
  DVE:  w  = max(-w', 6.1e-5)              tensor_scalar, dual op
  DVE:  l  = float(bits16(w))*C1 + C0      fused int16-bitcast log trick
  DVE:  s  = (g0 - g1) + (l1 - l0)         3x tensor_tensor
then sigmoids batched last (single Ln->Sigmoid table switch):
  ACT:  out = Sigmoid(s / TEMP), DMA out.

The second log uses the classic exponent+mantissa linear approximation
ln(w) ~= ln2*(bits16(w)/1024 - 15 + 0.043): max error 0.031 which enters
the sigmoid argument /TEMP -> ~3e-3.  The w clamp at the f16 min normal
(6.1e-5) doubles as a near-optimal estimator for u values that f16
rounds to exactly 1.0 (their true -ln(u) tail-mean is ~e^-1*2.4e-4).
End-to-end rel_fro vs f64 reference ~1.7e-3 (gate: 2e-2), dominated by
the f16 input quantization itself.

DMA order: u chunks run two ahead of g chunks on the sync ring, so the
ACT Ln pass (paced only by u arrivals) starts its table switch early and
the sigmoid phase overlaps the tail of the g-load/DVE pipeline.
"""

import math

import numpy as np

SZ = 8192
DEL_NUM = 2048
S = SZ - DEL_NUM               # 6144
E1 = S * DEL_NUM               # 12,582,912 dense block elements
E2 = DEL_NUM * (DEL_NUM - 1) // 2  # 2,096,128 triangular elements
E = E1 + E2                    # 14,679,040
NCORES = 8
CH = E // NCORES               # 1,834,880 elements per core
P = 128
FTOT = CH // P                 # 14,335 outputs per partition
SIZES = [2300] * 6 + [FTOT - 6 * 2300]   # chunk widths (last is the tail)
OFFS = [sum(SIZES[:i]) for i in range(len(SIZES))]
NCHUNK = len(SIZES)
TEMP = 10.0
EPS = 1e-20
WMIN16 = 6.1035156e-05         # f16 min normal: w floor + u->1 tail estimator
LN2 = math.log(2.0)
C1_16 = LN2 / 1024.0           # f16 bits: i = 1024*(e_biased + m)
C0_16 = -LN2 * (15.0 - 0.0430357)

_cache = {}


def _build(nrep=None):
    """Build the SPMD program.  nrep=None -> production single pass;
    nrep=N wraps the identical pass in a device-side For_i loop (timing
    instrument: one NEFF execution runs the pass N times back-to-back)."""
    import concourse.bacc as bacc
    import concourse.mybir as mybir
    import concourse.tile as tile

    f16 = mybir.dt.float16
    f32 = mybir.dt.float32
    i16 = mybir.dt.int16
    AF = mybir.ActivationFunctionType

    nc = bacc.Bacc(
        "TRN2", target_bir_lowering=False, debug=False, num_devices=NCORES
    )

    # Float activation biases require registered const APs.
    for val in (EPS,):
        t = nc.alloc_sbuf_tensor(f"const-f32-{val}", [128, 1], f32)
        nc.gpsimd.memset(t.ap(), val)
        nc.const_aps.aps[(f32, val)] = t.ap()
    nc.all_engine_barrier()

    u_ap = nc.dram_tensor("u", [P, 2 * FTOT], f16, kind="ExternalInput").ap()
    g_ap = nc.dram_tensor("gen", [P, 2 * FTOT], f16, kind="ExternalInput").ap()
    out_ap = nc.dram_tensor("out", [P, FTOT], f16, kind="ExternalOutput").ap()

    with tile.TileContext(nc) as tc:
        with tc.tile_pool(name="pool", bufs=2) as pool:

            def one_pass():
                uts = {}

                def load_u(i):
                    Fi, Oi = SIZES[i], OFFS[i]
                    uts[i] = pool.tile(
                        [P, 2 * Fi], f16, tag="u", bufs=3, name=f"ut{i}"
                    )
                    nc.sync.dma_start(uts[i][:], u_ap[:, 2 * Oi : 2 * (Oi + Fi)])

                load_u(0)
                load_u(1)
                st_list = []
                for i, (Fi, Oi) in enumerate(zip(SIZES, OFFS)):
                    if i + 2 < NCHUNK:
                        load_u(i + 2)
                    gt = pool.tile([P, 2 * Fi], f16, tag="g", bufs=2, name=f"gt{i}")
                    nc.sync.dma_start(gt[:], g_ap[:, 2 * Oi : 2 * (Oi + Fi)])
                    ut = uts.pop(i)

                    wt = pool.tile([P, 2 * Fi], f16, tag="w", bufs=2)
                    nc.scalar.activation(wt[:], ut[:], AF.Ln, bias=EPS)
                    nc.vector.tensor_scalar(
                        wt[:], wt[:], -1.0, WMIN16,
                        op0=mybir.AluOpType.mult, op1=mybir.AluOpType.max,
                    )
                    lt = pool.tile([P, 2 * Fi], f16, tag="l", bufs=2)
                    nc.vector.tensor_scalar(
                        lt[:], wt[:].bitcast(i16), C1_16, C0_16,
                        op0=mybir.AluOpType.mult, op1=mybir.AluOpType.add,
                    )

                    st = pool.tile([P, Fi], f16, tag="s", bufs=NCHUNK)
                    nc.vector.tensor_sub(st[:], gt[:, 0:Fi], gt[:, Fi : 2 * Fi])
                    dlt = pool.tile([P, Fi], f16, tag="dl", bufs=2)
                    nc.vector.tensor_sub(dlt[:], lt[:, Fi : 2 * Fi], lt[:, 0:Fi])
                    nc.vector.tensor_add(st[:], st[:], dlt[:])
                    st_list.append((st, Oi, Fi))

                # Sigmoids batched: one Ln->Sigmoid table switch per pass.
                for st, Oi, Fi in st_list:
                    nc.scalar.activation(st[:], st[:], AF.Sigmoid, scale=1.0 / TEMP)
                    nc.sync.dma_start(out_ap[:, Oi : Oi + Fi], st[:])

            if nrep is None:
                one_pass()
            else:
                with tc.For_i(0, nrep):
                    one_pass()

    nc.compile()
    return nc


def get_nc(nrep=None):
    if nrep not in _cache:
        _cache[nrep] = _build(nrep)
    return _cache[nrep]


def stage_core_inputs(arr: np.ndarray, core: int) -> np.ndarray:
    """Slice one core's [CH, 2] block and lay it out as [P, 2*FTOT] f16:
    within chunk i, component-0 values occupy the first Fi columns and
    component-1 the next Fi (unit-stride halves for the engines)."""
    a = arr[core * CH : (core + 1) * CH].astype(np.float16)
    a = a.reshape(P, FTOT, 2)
    out = np.empty((P, 2 * FTOT), np.float16)
    for Fi, Oi in zip(SIZES, OFFS):
        blk = a[:, Oi : Oi + Fi, :]
        out[:, 2 * Oi : 2 * Oi + Fi] = blk[:, :, 0]
        out[:, 2 * Oi + Fi : 2 * (Oi + Fi)] = blk[:, :, 1]
    return out


def run_cores(gen: np.ndarray, u: np.ndarray, trace: bool = False):
    """Run the SPMD kernel on flat [E, 2] inputs; returns (flat out [E], results obj)."""
    from concourse.bass_utils import run_bass_kernel_spmd

    nc = get_nc()
    in_maps = []
    for c in range(NCORES):
        in_maps.append(
            {"gen": stage_core_inputs(gen, c), "u": stage_core_inputs(u, c)}
        )
    kw = {}
    if trace:
        kw = {"trace": True, "trace_cores": list(range(NCORES)), "stitch_traces": True}
    res = run_bass_kernel_spmd(nc, in_maps, core_ids=list(range(NCORES)), **kw)
    out = np.concatenate(
        [np.asarray(r["out"]).astype(np.float32).reshape(-1) for r in res.results]
    )
    return out, res


def assemble(out: np.ndarray) -> np.ndarray:
    full = np.zeros((SZ, SZ), np.float32)
    a = out[:E1].reshape(S, DEL_NUM)
    full[:S, S:] = a
    full[S:, :S] = a.T
    ti, tj = np.triu_indices(DEL_NUM, k=1)
    b = np.zeros((DEL_NUM, DEL_NUM), np.float32)
    b[ti, tj] = out[E1:]
    full[S:, S:] = b + b.T
    return full


def kernel(gen_matrix=None, u=None, sz=None, del_num=None, **_ignored):
    gen = np.ascontiguousarray(np.asarray(gen_matrix, dtype=np.float32))
    uu = np.ascontiguousarray(np.asarray(u, dtype=np.float32))
    assert gen.shape == (E, 2) and uu.shape == (E, 2)
    out, _ = run_cores(gen, uu)
    return assemble(out)


# revision 4
# speedup vs baseline: 10.6439x; 1.0049x over previous
"""Gumbel-softmax sample + symmetric scatter kernel for 8 trn2 NeuronCores.

Math: out[e] = sigmoid(((g0 - g1) + (gum0 - gum1)) / TEMP) with
gum_k = -log(-log(u_k + EPS) + EPS).  The scatter target is fully
deterministic: part 1 (first S*DEL_NUM elements) is a dense [S, DEL_NUM]
block at matrix[0:S, S:SZ]; part 2 is the strict upper triangle of the
bottom-right [DEL_NUM, DEL_NUM] block.  Output = matrix + matrix.T.

Device: each core computes a contiguous 1/8 of the E sigmoid values
(memory-bound elementwise map).  Host places the values into the
symmetric [SZ, SZ] output.

v3 design (per core, per chunk; all engine operands f16 so DVE runs in
its 2x/4x perf modes and HBM traffic is halved):
  ACT:  w' = Ln(u + EPS)                   (f16 in -> f16 out)
  DVE:  w  = max(-w', 6.1e-5)              tensor_scalar, dual op
  DVE:  l  = float(bits16(w))*C1 + C0      fused int16-bitcast log trick
  DVE:  s  = (g0 - g1) + (l1 - l0)         3x tensor_tensor
then sigmoids batched last (single Ln->Sigmoid table switch):
  ACT:  out = Sigmoid(s / TEMP), DMA out.

The second log uses the classic exponent+mantissa linear approximation
ln(w) ~= ln2*(bits16(w)/1024 - 15 + 0.043): max error 0.031 which enters
the sigmoid argument /TEMP -> ~3e-3.  The w clamp at the f16 min normal
(6.1e-5) doubles as a near-optimal estimator for u values that f16
rounds to exactly 1.0 (their true -ln(u) tail-mean is ~e^-1*2.4e-4).
End-to-end rel_fro vs f64 reference ~1.7e-3 (gate: 2e-2), dominated by
the f16 input quantization itself.

DMA order: u chunks run two ahead of g chunks on the sync ring, so the
ACT Ln pass (paced only by u arrivals) starts its table switch early and
the sigmoid phase overlaps the tail of the g-load/DVE pipeline.
"""

import math

import numpy as np

SZ = 8192
DEL_NUM = 2048
S = SZ - DEL_NUM               # 6144
E1 = S * DEL_NUM               # 12,582,912 dense block elements
E2 = DEL_NUM * (DEL_NUM - 1) // 2  # 2,096,128 triangular elements
E = E1 + E2                    # 14,679,040
NCORES = 8
CH = E // NCORES               # 1,834,880 elements per core
P = 128
FTOT = CH // P                 # 14,335 outputs per partition
# Chunk widths: small first chunk (fast pipeline fill), large middle
# (DMA efficiency), small tail (short drain) — best of the HW-verified
# TimelineSim sweep.
SIZES = [1500, 2500, 3000, 3000, 2300, 1500, FTOT - 13800]
OFFS = [sum(SIZES[:i]) for i in range(len(SIZES))]
NCHUNK = len(SIZES)
TEMP = 10.0
EPS = 1e-20
WMIN16 = 6.1035156e-05         # f16 min normal: w floor + u->1 tail estimator
LN2 = math.log(2.0)
C1_16 = LN2 / 1024.0           # f16 bits: i = 1024*(e_biased + m)
C0_16 = -LN2 * (15.0 - 0.0430357)

_cache = {}


def _build(nrep=None):
    """Build the SPMD program.  nrep=None -> production single pass;
    nrep=N wraps the identical pass in a device-side For_i loop (timing
    instrument: one NEFF execution runs the pass N times back-to-back)."""
    import concourse.bacc as bacc
    import concourse.mybir as mybir
    import concourse.tile as tile

    f16 = mybir.dt.float16
    f32 = mybir.dt.float32
    i16 = mybir.dt.int16
    AF = mybir.ActivationFunctionType

    nc = bacc.Bacc(
        "TRN2", target_bir_lowering=False, debug=False, num_devices=NCORES
    )

    # Float activation biases require registered const APs.
    for val in (EPS,):
        t = nc.alloc_sbuf_tensor(f"const-f32-{val}", [128, 1], f32)
        nc.gpsimd.memset(t.ap(), val)
        nc.const_aps.aps[(f32, val)] = t.ap()
    nc.all_engine_barrier()

    u_ap = nc.dram_tensor("u", [P, 2 * FTOT], f16, kind="ExternalInput").ap()
    g_ap = nc.dram_tensor("gen", [P, 2 * FTOT], f16, kind="ExternalInput").ap()
    out_ap = nc.dram_tensor("out", [P, FTOT], f16, kind="ExternalOutput").ap()

    with tile.TileContext(nc) as tc:
        with tc.tile_pool(name="pool", bufs=2) as pool:

            def one_pass():
                uts = {}

                def load_u(i):
                    Fi, Oi = SIZES[i], OFFS[i]
                    uts[i] = pool.tile(
                        [P, 2 * Fi], f16, tag="u", bufs=3, name=f"ut{i}"
                    )
                    nc.sync.dma_start(uts[i][:], u_ap[:, 2 * Oi : 2 * (Oi + Fi)])

                load_u(0)
                load_u(1)
                st_list = []
                for i, (Fi, Oi) in enumerate(zip(SIZES, OFFS)):
                    if i + 2 < NCHUNK:
                        load_u(i + 2)
                    gt = pool.tile([P, 2 * Fi], f16, tag="g", bufs=2, name=f"gt{i}")
                    nc.sync.dma_start(gt[:], g_ap[:, 2 * Oi : 2 * (Oi + Fi)])
                    ut = uts.pop(i)

                    wt = pool.tile([P, 2 * Fi], f16, tag="w", bufs=2)
                    nc.scalar.activation(wt[:], ut[:], AF.Ln, bias=EPS)
                    nc.vector.tensor_scalar(
                        wt[:], wt[:], -1.0, WMIN16,
                        op0=mybir.AluOpType.mult, op1=mybir.AluOpType.max,
                    )
                    lt = pool.tile([P, 2 * Fi], f16, tag="l", bufs=2)
                    nc.vector.tensor_scalar(
                        lt[:], wt[:].bitcast(i16), C1_16, C0_16,
                        op0=mybir.AluOpType.mult, op1=mybir.AluOpType.add,
                    )

                    st = pool.tile([P, Fi], f16, tag="s", bufs=NCHUNK)
                    nc.vector.tensor_sub(st[:], gt[:, 0:Fi], gt[:, Fi : 2 * Fi])
                    dlt = pool.tile([P, Fi], f16, tag="dl", bufs=2)
                    nc.vector.tensor_sub(dlt[:], lt[:, Fi : 2 * Fi], lt[:, 0:Fi])
                    nc.vector.tensor_add(st[:], st[:], dlt[:])
                    st_list.append((st, Oi, Fi))

                # Sigmoids batched: one Ln->Sigmoid table switch per pass.
                for st, Oi, Fi in st_list:
                    nc.scalar.activation(st[:], st[:], AF.Sigmoid, scale=1.0 / TEMP)
                    nc.sync.dma_start(out_ap[:, Oi : Oi + Fi], st[:])

            if nrep is None:
                one_pass()
            else:
                with tc.For_i(0, nrep):
                    one_pass()

    nc.compile()
    return nc


def get_nc(nrep=None):
    if nrep not in _cache:
        _cache[nrep] = _build(nrep)
    return _cache[nrep]


def stage_core_inputs(arr: np.ndarray, core: int) -> np.ndarray:
    """Slice one core's [CH, 2] block and lay it out as [P, 2*FTOT] f16:
    within chunk i, component-0 values occupy the first Fi columns and
    component-1 the next Fi (unit-stride halves for the engines)."""
    a = arr[core * CH : (core + 1) * CH].astype(np.float16)
    a = a.reshape(P, FTOT, 2)
    out = np.empty((P, 2 * FTOT), np.float16)
    for Fi, Oi in zip(SIZES, OFFS):
        blk = a[:, Oi : Oi + Fi, :]
        out[:, 2 * Oi : 2 * Oi + Fi] = blk[:, :, 0]
        out[:, 2 * Oi + Fi : 2 * (Oi + Fi)] = blk[:, :, 1]
    return out


def run_cores(gen: np.ndarray, u: np.ndarray, trace: bool = False):
    """Run the SPMD kernel on flat [E, 2] inputs; returns (flat out [E], results obj)."""
    from concourse.bass_utils import run_bass_kernel_spmd

    nc = get_nc()
    in_maps = []
    for c in range(NCORES):
        in_maps.append(
            {"gen": stage_core_inputs(gen, c), "u": stage_core_inputs(u, c)}
        )
    kw = {}
    if trace:
        kw = {"trace": True, "trace_cores": list(range(NCORES)), "stitch_traces": True}
    res = run_bass_kernel_spmd(nc, in_maps, core_ids=list(range(NCORES)), **kw)
    out = np.concatenate(
        [np.asarray(r["out"]).astype(np.float32).reshape(-1) for r in res.results]
    )
    return out, res


def assemble(out: np.ndarray) -> np.ndarray:
    full = np.zeros((SZ, SZ), np.float32)
    a = out[:E1].reshape(S, DEL_NUM)
    full[:S, S:] = a
    full[S:, :S] = a.T
    ti, tj = np.triu_indices(DEL_NUM, k=1)
    b = np.zeros((DEL_NUM, DEL_NUM), np.float32)
    b[ti, tj] = out[E1:]
    full[S:, S:] = b + b.T
    return full


def kernel(gen_matrix=None, u=None, sz=None, del_num=None, **_ignored):
    gen = np.ascontiguousarray(np.asarray(gen_matrix, dtype=np.float32))
    uu = np.ascontiguousarray(np.asarray(u, dtype=np.float32))
    assert gen.shape == (E, 2) and uu.shape == (E, 2)
    out, _ = run_cores(gen, uu)
    return assemble(out)


# revision 5
# speedup vs baseline: 11.0901x; 1.0419x over previous
"""Gumbel-softmax sample + symmetric scatter kernel for 8 trn2 NeuronCores.

Math: out[e] = sigmoid(((g0 - g1) + (gum0 - gum1)) / TEMP) with
gum_k = -log(-log(u_k + EPS) + EPS).  The scatter target is fully
deterministic: part 1 (first S*DEL_NUM elements) is a dense [S, DEL_NUM]
block at matrix[0:S, S:SZ]; part 2 is the strict upper triangle of the
bottom-right [DEL_NUM, DEL_NUM] block.  Output = matrix + matrix.T.

Device: each core computes a contiguous 1/8 of the E sigmoid values
(memory-bound elementwise map).  Host places the values into the
symmetric [SZ, SZ] output.

v3 design (per core, per chunk; all engine operands f16 so DVE runs in
its 2x/4x perf modes and HBM traffic is halved):
  ACT:  w' = Ln(u + EPS)                   (f16 in -> f16 out)
  DVE:  w  = max(-w', 6.1e-5)              tensor_scalar, dual op
  DVE:  l  = float(bits16(w))*C1 + C0      fused int16-bitcast log trick
  DVE:  s  = (g0 - g1) + (l1 - l0)         3x tensor_tensor
then sigmoids batched last (single Ln->Sigmoid table switch):
  ACT:  out = Sigmoid(s / TEMP), DMA out.

The second log uses the classic exponent+mantissa linear approximation
ln(w) ~= ln2*(bits16(w)/1024 - 15 + 0.043): max error 0.031 which enters
the sigmoid argument /TEMP -> ~3e-3.  The w clamp at the f16 min normal
(6.1e-5) doubles as a near-optimal estimator for u values that f16
rounds to exactly 1.0 (their true -ln(u) tail-mean is ~e^-1*2.4e-4).
End-to-end rel_fro vs f64 reference ~1.7e-3 (gate: 2e-2), dominated by
the f16 input quantization itself.

DMA order: u chunks run two ahead of g chunks on the sync ring, so the
ACT Ln pass (paced only by u arrivals) starts its table switch early and
the sigmoid phase overlaps the tail of the g-load/DVE pipeline.
"""

import math

import numpy as np

SZ = 8192
DEL_NUM = 2048
S = SZ - DEL_NUM               # 6144
E1 = S * DEL_NUM               # 12,582,912 dense block elements
E2 = DEL_NUM * (DEL_NUM - 1) // 2  # 2,096,128 triangular elements
E = E1 + E2                    # 14,679,040
NCORES = 8
CH = E // NCORES               # 1,834,880 elements per core
P = 128
FTOT = CH // P                 # 14,335 outputs per partition
# Chunk widths: small first chunk (fast pipeline fill), large middle
# (DMA efficiency), small tail (short drain) — best of the HW-verified
# TimelineSim sweep.
SIZES = [1500, 2500, 3000, 3000, 2300, 1500, FTOT - 13800]
OFFS = [sum(SIZES[:i]) for i in range(len(SIZES))]
NCHUNK = len(SIZES)
TEMP = 10.0
EPS = 1e-20
WMIN16 = 6.1035156e-05         # f16 min normal: w floor + u->1 tail estimator
LN2 = math.log(2.0)
C1_16 = LN2 / 1024.0           # f16 bits: i = 1024*(e_biased + m)
C0_16 = -LN2 * (15.0 - 0.0430357)

_cache = {}


def _build(nrep=None):
    """Build the SPMD program.  nrep=None -> production single pass;
    nrep=N wraps the identical pass in a device-side For_i loop (timing
    instrument: one NEFF execution runs the pass N times back-to-back)."""
    import concourse.bacc as bacc
    import concourse.mybir as mybir
    import concourse.tile as tile

    f16 = mybir.dt.float16
    f32 = mybir.dt.float32
    i16 = mybir.dt.int16
    AF = mybir.ActivationFunctionType

    nc = bacc.Bacc(
        "TRN2", target_bir_lowering=False, debug=False, num_devices=NCORES
    )

    # Float activation biases require registered const APs.
    for val in (EPS,):
        t = nc.alloc_sbuf_tensor(f"const-f32-{val}", [128, 1], f32)
        nc.gpsimd.memset(t.ap(), val)
        nc.const_aps.aps[(f32, val)] = t.ap()
    nc.all_engine_barrier()

    u_ap = nc.dram_tensor("u", [P, 2 * FTOT], f16, kind="ExternalInput").ap()
    g_ap = nc.dram_tensor("gen", [P, 2 * FTOT], f16, kind="ExternalInput").ap()
    out_ap = nc.dram_tensor("out", [P, FTOT], f16, kind="ExternalOutput").ap()

    with tile.TileContext(nc) as tc:
        with tc.tile_pool(name="pool", bufs=2) as pool:

            def one_pass():
                uts = {}

                def load_u(i):
                    Fi, Oi = SIZES[i], OFFS[i]
                    uts[i] = pool.tile(
                        [P, 2 * Fi], f16, tag="u", bufs=4, name=f"ut{i}"
                    )
                    nc.sync.dma_start(uts[i][:], u_ap[:, 2 * Oi : 2 * (Oi + Fi)])

                load_u(0)
                load_u(1)
                st_list = []
                for i, (Fi, Oi) in enumerate(zip(SIZES, OFFS)):
                    if i + 2 < NCHUNK:
                        load_u(i + 2)
                    gt = pool.tile([P, 2 * Fi], f16, tag="g", bufs=4, name=f"gt{i}")
                    nc.sync.dma_start(gt[:], g_ap[:, 2 * Oi : 2 * (Oi + Fi)])
                    ut = uts.pop(i)

                    wt = pool.tile([P, 2 * Fi], f16, tag="w", bufs=2)
                    nc.scalar.activation(wt[:], ut[:], AF.Ln, bias=EPS)
                    nc.vector.tensor_scalar(
                        wt[:], wt[:], -1.0, WMIN16,
                        op0=mybir.AluOpType.mult, op1=mybir.AluOpType.max,
                    )
                    lt = pool.tile([P, 2 * Fi], f16, tag="l", bufs=2)
                    nc.vector.tensor_scalar(
                        lt[:], wt[:].bitcast(i16), C1_16, C0_16,
                        op0=mybir.AluOpType.mult, op1=mybir.AluOpType.add,
                    )

                    st = pool.tile([P, Fi], f16, tag="s", bufs=NCHUNK, name=f"st{i}")
                    nc.vector.tensor_sub(st[:], gt[:, 0:Fi], gt[:, Fi : 2 * Fi])
                    dlt = pool.tile([P, Fi], f16, tag="dl", bufs=2)
                    nc.vector.tensor_sub(dlt[:], lt[:, Fi : 2 * Fi], lt[:, 0:Fi])
                    nc.vector.tensor_add(st[:], st[:], dlt[:])
                    st_list.append((st, Oi, Fi))

                # Sigmoids batched: one Ln->Sigmoid table switch per pass.
                for st, Oi, Fi in st_list:
                    nc.scalar.activation(st[:], st[:], AF.Sigmoid, scale=1.0 / TEMP)
                    nc.sync.dma_start(out_ap[:, Oi : Oi + Fi], st[:])

            if nrep is None:
                one_pass()
            else:
                with tc.For_i(0, nrep):
                    one_pass()

    nc.compile()
    return nc


def get_nc(nrep=None):
    if nrep not in _cache:
        _cache[nrep] = _build(nrep)
    return _cache[nrep]


def stage_core_inputs(arr: np.ndarray, core: int) -> np.ndarray:
    """Slice one core's [CH, 2] block and lay it out as [P, 2*FTOT] f16:
    within chunk i, component-0 values occupy the first Fi columns and
    component-1 the next Fi (unit-stride halves for the engines)."""
    a = arr[core * CH : (core + 1) * CH].astype(np.float16)
    a = a.reshape(P, FTOT, 2)
    out = np.empty((P, 2 * FTOT), np.float16)
    for Fi, Oi in zip(SIZES, OFFS):
        blk = a[:, Oi : Oi + Fi, :]
        out[:, 2 * Oi : 2 * Oi + Fi] = blk[:, :, 0]
        out[:, 2 * Oi + Fi : 2 * (Oi + Fi)] = blk[:, :, 1]
    return out


def run_cores(gen: np.ndarray, u: np.ndarray, trace: bool = False):
    """Run the SPMD kernel on flat [E, 2] inputs; returns (flat out [E], results obj)."""
    from concourse.bass_utils import run_bass_kernel_spmd

    nc = get_nc()
    in_maps = []
    for c in range(NCORES):
        in_maps.append(
            {"gen": stage_core_inputs(gen, c), "u": stage_core_inputs(u, c)}
        )
    kw = {}
    if trace:
        kw = {"trace": True, "trace_cores": list(range(NCORES)), "stitch_traces": True}
    res = run_bass_kernel_spmd(nc, in_maps, core_ids=list(range(NCORES)), **kw)
    out = np.concatenate(
        [np.asarray(r["out"]).astype(np.float32).reshape(-1) for r in res.results]
    )
    return out, res


def assemble(out: np.ndarray) -> np.ndarray:
    full = np.zeros((SZ, SZ), np.float32)
    a = out[:E1].reshape(S, DEL_NUM)
    full[:S, S:] = a
    full[S:, :S] = a.T
    ti, tj = np.triu_indices(DEL_NUM, k=1)
    b = np.zeros((DEL_NUM, DEL_NUM), np.float32)
    b[ti, tj] = out[E1:]
    full[S:, S:] = b + b.T
    return full


def kernel(gen_matrix=None, u=None, sz=None, del_num=None, **_ignored):
    gen = np.ascontiguousarray(np.asarray(gen_matrix, dtype=np.float32))
    uu = np.ascontiguousarray(np.asarray(u, dtype=np.float32))
    assert gen.shape == (E, 2) and uu.shape == (E, 2)
    out, _ = run_cores(gen, uu)
    return assemble(out)
